# revision 1
# baseline (speedup 1.0000x reference)
"""Fused FADocker coordinate-update kernel for 8 Trainium2 NeuronCores.

Math: for the reference
    gate[b,i,j,a] = relu(hw[b,i,:] + hu[b,j,:]) @ Tx_w[a,:] + Tx_b[a]
    f[b,i,a,c]    = sum_j gate * (X[b,i,a,c] - X[b,j,a,c]) * m[b,j]
                  = X[b,i,a,c] * G[b,i,a] - S[b,i,a,c]
with
    S'[b,(a,c'),i] = sum_j gate_raw[b,i,j,a] * Xm4[b,j,a,c']
    Xm4[b,j,a,c']  = X[b,j,a,c]*m[b,j] for c'<3,  m[b,j] for c'=3
The j-sum is folded into PSUM accumulation: per (j, h-half) one matmul with
stationary weights W_j[h,(a,c')] = TxT[h,a]*Xm4[j,a,c'] (built on host, bf16)
against the streamed rhs relu(hwT_half + huT_half[:, j]) of shape [128, 384].
All relu tiles are produced by the DVE (fused add+max tensor_scalar, ~88ns
per [128,384] bf16 tile on HW); PE consumes them at ~170ns per matmul and is
the bottleneck. Sharding: core c -> batch b=c//4, j-range = (c%4)*96..+96.
Host sums the 8 partial [56, 384] outputs and applies the (i-independent)
Tx_b correction, divide, clip, and residual add.
"""

import contextlib
from concurrent.futures import ThreadPoolExecutor

import numpy as np
import ml_dtypes

import concourse.bass as bass
import concourse.tile as tile
from concourse import bacc, mybir
from concourse.bass import RegisterHandles, make_scalar_value
from concourse.bass_utils import run_bass_kernel_spmd

B, L, H, A = 2, 384, 256, 14
NCORES = 8
JSHARD = L // 4          # 96 j's per core
CHUNK_J = 32             # j's per W-chunk DMA
NCHUNK = JSHARD // CHUNK_J
AC = A * 4               # 56 = (a, c') columns
P = 128
PKCOLS = L + JSHARD + H + H + 1   # packed const columns: 993

F32 = mybir.dt.float32
BF16 = mybir.dt.bfloat16

_cached = {}


def _build_program(reps=1, dyn_loop=False):
    key = ("nc", reps, dyn_loop)
    if key in _cached:
        return _cached[key]

    nc = bacc.Bacc("TRN2", target_bir_lowering=False, debug=False)

    pk_d = nc.dram_tensor("pk", [2, P, PKCOLS], BF16, kind="ExternalInput").ap()
    # Wt[half] is W[b, half*128+hh, j_local, (a,c')] flattened: the natural
    # (contiguous) layout of the host-side broadcast product, so the host
    # does no transposes; the per-chunk DMAs below are strided slices.
    Wt_d = nc.dram_tensor("Wt", [2, P, JSHARD * AC], BF16,
                          kind="ExternalInput").ap()
    if dyn_loop:
        ln_d = nc.dram_tensor("ln", [1, 1], mybir.dt.int32,
                              kind="ExternalInput").ap()
    Sp_d = nc.dram_tensor("Sp", [AC, L], F32, kind="ExternalOutput").ap()

    with tile.TileContext(nc, trace_sim=False) as tc:
        with (
            tc.tile_pool(name="const", bufs=1) as cpool,
            tc.tile_pool(name="r", bufs=12) as rpool,
            tc.tile_pool(name="psum", bufs=2, space="PSUM") as pspool,
            tc.tile_pool(name="psum_s", bufs=1, space="PSUM") as pspool_s,
        ):
            # pk (projection inputs) first, split across the HWDGE and ACT
            # SWDGE queues; W chunks on the gpsimd (SWDGE) queue with ~4us
            # of slack before first use.
            pk_sb = []
            for k, eng in ((0, nc.sync), (1, nc.scalar)):
                t = cpool.tile([P, PKCOLS], BF16, tag=f"pk{k}")
                eng.dma_start(t[:], pk_d[k])
                pk_sb.append(t)
            wt_sb = [[], []]
            for half in range(2):
                for ck in range(NCHUNK):
                    t = cpool.tile([P, CHUNK_J * AC], BF16, tag=f"wt{half}_{ck}")
                    c0 = ck * CHUNK_J * AC
                    nc.gpsimd.dma_start(t[:], Wt_d[half][:, c0:c0 + CHUNK_J * AC])
                    wt_sb[half].append(t)
            o_hT = 0
            o_hTj = o_hT + L
            o_Wx = o_hTj + JSHARD
            o_Ux = o_Wx + H
            o_bc = o_Ux + H
            hT_sb = [t[:, o_hT:o_hT + L] for t in pk_sb]
            hTj_sb = [t[:, o_hTj:o_hTj + JSHARD] for t in pk_sb]
            WxT_sb = [t[:, o_Wx:o_Wx + H] for t in pk_sb]
            UxT_sb = [t[:, o_Ux:o_Ux + H] for t in pk_sb]
            bcomb_sb = []
            for k in range(2):
                bc = cpool.tile([P, 1], F32, tag=f"bc{k}")
                nc.vector.tensor_copy(bc[:], pk_sb[k][:, o_bc:o_bc + 1])
                bcomb_sb.append(bc)

            if dyn_loop:
                ln_t = cpool.tile([1, 1], mybir.dt.int32, tag="ln")
                nc.sync.dma_start(ln_t[:], ln_d[:])
                regs = []
                for e in mybir.ALL_ENGINES:
                    r = nc.alloc_register(e, f"lnreg_{e.name}")
                    nc.engines[e].reg_load(r, ln_t[0:1, 0:1])
                    regs.append(r)
                end_val = make_scalar_value(RegisterHandles(regs),
                                            min_val=0, max_val=1 << 20)
                loop_cm = tc.For_i(0, end_val, 1)
            else:
                loop_cm = contextlib.nullcontext()
            with loop_cm:
                for rep in range(reps):
                    _emit_body(nc, tc, hT_sb, hTj_sb, WxT_sb, UxT_sb,
                               bcomb_sb, wt_sb, Sp_d, cpool, rpool, pspool,
                               pspool_s)

    nc.compile()
    _cached[key] = nc
    return nc


def _emit_body(nc, tc, hT_sb, hTj_sb, WxT_sb, UxT_sb, bcomb_sb, wt_sb,
               Sp_d, cpool, rpool, pspool, pspool_s):
    # Projections: hwT = Wx_w @ hT (bf16, ACT copies), huT = Ux_w @ hTj + bcomb
    hwT_sb = [None, None]
    huT_sb = [None, None]

    def proj(m):
        ps = pspool.tile([P, L], F32, tag="proj_w")
        for k in range(2):
            nc.tensor.matmul(ps[:], lhsT=WxT_sb[k][:, m * P:(m + 1) * P],
                             rhs=hT_sb[k][:], start=(k == 0), stop=(k == 1))
        hw = cpool.tile([P, L], BF16, tag=f"hwT{m}")
        nc.scalar.copy(hw[:], ps[:])
        hwT_sb[m] = hw

        ps2 = pspool.tile([P, JSHARD], F32, tag="proj_u")
        for k in range(2):
            nc.tensor.matmul(ps2[:], lhsT=UxT_sb[k][:, m * P:(m + 1) * P],
                             rhs=hTj_sb[k][:], start=(k == 0), stop=(k == 1))
        hu = cpool.tile([P, JSHARD], F32, tag=f"huT{m}")
        nc.scalar.activation(hu[:], ps2[:],
                             mybir.ActivationFunctionType.Identity,
                             bias=bcomb_sb[m][:], scale=1.0)
        huT_sb[m] = hu

    proj(0)

    # Main loop: accumulate S'[(a,c'), i] over all local (j, half). The
    # half-0 stream only needs the m=0 projections; m=1 is emitted a few
    # iterations in so PE starts the main stream as early as possible.
    S_ps = pspool_s.tile([AC, L], F32, tag="S")
    idx = 0
    nmm = 2 * JSHARD
    hl = L // 2
    for half in range(2):
        for j in range(JSHARD):
            if half == 0 and j == 8:
                proj(1)
            ck, jj = divmod(j, CHUNK_J)
            col = jj * AC
            lhsT = wt_sb[half][ck][:, col:col + AC]
            r = rpool.tile([P, L], BF16, tag="r")
            if idx % 3 == 2:
                nc.scalar.activation(
                    r[:], hwT_sb[half][:], mybir.ActivationFunctionType.Relu,
                    bias=huT_sb[half][:, j:j + 1], scale=1.0)
            else:
                nc.vector.tensor_scalar(
                    r[:], hwT_sb[half][:], huT_sb[half][:, j:j + 1], 0.0,
                    mybir.AluOpType.add, mybir.AluOpType.max)
            nc.tensor.matmul(S_ps[:], lhsT=lhsT,
                             rhs=r[:], start=(idx == 0), stop=(idx == nmm - 1))
            idx += 1

    # Tail: copy the two column halves on ACT and DVE in parallel, each
    # followed by its own DMA.
    out_sb = cpool.tile([AC, L], F32, tag="out")
    hl = L // 2
    nc.scalar.copy(out_sb[:, 0:hl], S_ps[:, 0:hl])
    nc.vector.tensor_copy(out_sb[:, hl:L], S_ps[:, hl:L])
    nc.sync.dma_start(Sp_d[:, 0:hl], out_sb[:, 0:hl])
    nc.gpsimd.dma_start(Sp_d[:, hl:L], out_sb[:, hl:L])


def _prepare_in_maps(h, X, mask, Wx_w, Wx_b, Ux_w, Ux_b, Tx_w, Tx_b):
    m = mask.astype(np.float32)                                   # (B, L)
    hT = np.ascontiguousarray(h.transpose(0, 2, 1)).astype(ml_dtypes.bfloat16)
    WxT = np.ascontiguousarray(Wx_w.T).astype(ml_dtypes.bfloat16)
    UxT = np.ascontiguousarray(Ux_w.T).astype(ml_dtypes.bfloat16)
    bcomb = (Wx_b + Ux_b).astype(ml_dtypes.bfloat16).reshape(H, 1)

    # Xm4[b, j, a, c'] with c'=3 holding m
    Xm4 = np.empty((B, L, A, 4), np.float32)
    Xm4[..., :3] = X * m[:, :, None, None]
    Xm4[..., 3] = m[:, :, None]

    # TxT_ac[h, (a,c')] = Tx_w.T repeated over c'
    TxT_ac = np.repeat(Tx_w.T.astype(np.float32), 4, axis=1)      # (H, 56)
    Xm4_ac = Xm4.reshape(B, L, AC)

    def build_core(c):
        b, q = divmod(c, 4)
        j0 = q * JSHARD
        # W[h, j, ac] = TxT_ac[h, ac] * Xm4_ac[b, j0+j, ac]; the bf16 cast
        # lands directly in the device layout [half, hh, j*ac] (no copies).
        Wc = (TxT_ac[:, None, :] * Xm4_ac[b, None, j0:j0 + JSHARD, :]
              ).astype(ml_dtypes.bfloat16).reshape(2, P, JSHARD * AC)
        pk = np.empty((2, P, PKCOLS), ml_dtypes.bfloat16)
        for k in range(2):
            sl = slice(k * P, (k + 1) * P)
            pk[k, :, 0:L] = hT[b][sl]
            pk[k, :, L:L + JSHARD] = hT[b][sl, j0:j0 + JSHARD]
            pk[k, :, L + JSHARD:L + JSHARD + H] = WxT[sl]
            pk[k, :, L + JSHARD + H:L + JSHARD + 2 * H] = UxT[sl]
            pk[k, :, L + JSHARD + 2 * H:] = bcomb[sl]
        return {"pk": pk, "Wt": Wc}

    with ThreadPoolExecutor(max_workers=NCORES) as ex:
        in_maps = list(ex.map(build_core, range(NCORES)))
    return in_maps, m, Xm4


def _epilogue(results, X, m, Xm4, Tx_b):
    S4 = np.zeros((B, A, 4, L), np.float32)
    for c in range(NCORES):
        S4[c // 4] += results[c]["Sp"].reshape(A, 4, L)
    Sraw = S4.transpose(0, 3, 1, 2)                               # (B, L, A, 4)
    CX = Xm4.sum(axis=1)                                          # (B, A, 4)
    S_tot = Sraw + Tx_b[None, None, :, None] * CX[:, None]        # (B, L, A, 4)
    G = S_tot[..., 3]                                             # (B, L, A)
    S3 = S_tot[..., :3]                                           # (B, L, A, 3)
    denom = 1e-6 + m.sum(axis=1)[:, None, None, None]
    f = (X * G[..., None] - S3) / denom
    return (X + np.clip(f, -20.0, 20.0)).astype(np.float32)


def _run(trace=False, **inputs):
    inputs = {k: np.asarray(v) for k, v in inputs.items()}
    X = inputs["X"].astype(np.float32)
    nc = _build_program()
    in_maps, m, Xm4 = _prepare_in_maps(**inputs)
    for attempt in range(3):
        res = run_bass_kernel_spmd(nc, in_maps, core_ids=list(range(NCORES)),
                                   trace=trace)
        # Inputs are finite, so every partial must be finite; a non-finite
        # value indicates a transient device fault -> retry.
        if all(np.isfinite(r["Sp"]).all() for r in res.results):
            break
    out = _epilogue(res.results, X, m, Xm4, inputs["Tx_b"].astype(np.float32))
    return out, res


def kernel(**inputs):
    out, _ = _run(trace=False, **inputs)
    return out

